# revision 31
# baseline (speedup 1.0000x reference)
"""LCNN conv2d kernel for Trainium2 (8 NeuronCores, batch-sharded).

Math: out[b,o,h,w] = sum_d Wmat[o,d] * conv2d(x, dictionary)[b,d,h,w]
where Wmat is the scatter-add of lookup_coefficients into [O, D].

Device strategy (per core, 2 batches), all-bf16, all matmuls K=64
row-group tiled so pairs run concurrently in the PE array and there are
no full-array<->row-group transitions (each costs ~250 ns):
 - input buffer XX [128, F+PW]: rows 0:64 = padded x, rows 64:128 = x
   shifted by (1 row + 1 col).  Tile t0 of a pair computes its 9 conv
   taps on PE rows 0:64 (plain view offsets), tile t1 concurrently on
   rows 64:128 (offsets shifted (-1,-1); the (2,0) tap uses a -1-column
   view of the same buffer).  4.5 effective PE slots per tile, half the
   input HBM traffic of 4-way duplication (4.9 MB vs 9.8 MB per core).
 - stage 2 [O=256, D->128] also K=64-split: two slots per tile, each
   running (o-half, d-lo) on rows 0:64 concurrently with the other
   o-half x d-hi on rows 64:128, cross-scheduled over two PSUM banks so
   no bank is written by two matmuls at once.
 - outputs staged as bf16, DMA'd once per tile-pair (b=0 pairs on the
   gpsimd SWDGE ring, b=1 on the sync HWDGE ring); weights go on the
   scalar HWDGE ring so they land in parallel with the first x chunks;
   host upcasts to f32.
"""
import os
import sys

for _p in ("/opt/trn_rl_repo", "/root/.axon_site/_ro/trn_rl_repo"):
    if os.path.isdir(_p) and _p not in sys.path:
        sys.path.insert(0, _p)

import ml_dtypes
import numpy as np
from contextlib import ExitStack

from concourse import bacc, mybir, tile
from concourse.bass_utils import run_bass_kernel_spmd

# problem shapes (hardcoded per contract)
B, CIN, H, W = 16, 64, 96, 96
D, O = 100, 256
DP = 128                   # D padded to full PE width
NCORES = 8
BPC = B // NCORES          # batches per core
PH, PW = H + 2, W + 2      # zero-padded spatial
F = BPC * PH * PW          # per-partition x extent
FX = F + PW                # + tail pad so the (-1 col) view stays in-bounds
R = 4                      # output rows per matmul tile
NT = H // R                # h-tiles per batch
N = R * W                  # matmul free size (384)
PB = 512                   # psum bank stride (f32 elems)
TAPS = [(kh, kw) for kh in range(3) for kw in range(3)]
STAGE2_SWAP = True         # row-group stage2 via c1 partition-swap DMA
                           # (swaps get the sync ring to themselves;
                           # outputs move to the gpsimd ring)
WEIGHTS_RING = "scalar"    # which engine ring loads wst/wm
bf16 = mybir.dt.bfloat16
f32 = mybir.dt.float32

_NC_CACHE = {}


def _build():
    nc = bacc.Bacc(None, target_bir_lowering=False, debug=False)
    xx = nc.declare_dram_parameter("xx", [128, FX], bf16, isOutput=False)
    wst = nc.declare_dram_parameter("wst", [128, 9 * DP], bf16, isOutput=False)
    wm = nc.declare_dram_parameter("wm", [DP, 2 * O], bf16, isOutput=False)
    out = nc.declare_dram_parameter("out", [BPC, O, H, W], bf16, isOutput=True)

    with tile.TileContext(nc) as tc, ExitStack() as ctx:
        sb = ctx.enter_context(tc.tile_pool(name="sb", bufs=1))
        c1p = ctx.enter_context(tc.tile_pool(name="c1p", bufs=6))
        c1sp = ctx.enter_context(tc.tile_pool(name="c1sp", bufs=6))
        stgp = ctx.enter_context(tc.tile_pool(name="stgp", bufs=6))
        pcp = ctx.enter_context(tc.tile_pool(name="pcp", bufs=2, space="PSUM"))
        pop = ctx.enter_context(tc.tile_pool(name="pop", bufs=2, space="PSUM"))

        XX = sb.tile([128, FX], bf16)
        wst_s = sb.tile([128, 9 * DP], bf16)
        wm_s = sb.tile([DP, 2 * O], bf16)
        # weights via the scalar HWDGE ring: lands in parallel with the
        # sync ring's first x chunks, well before the first real matmul
        # startup is gated by CUMULATIVE early DMA bytes (shared SDMA
        # bandwidth), so order the sync ring by first-use: tap 0-2 slabs,
        # the 12 rows pair 0 reads, the remaining slabs, then the rest.
        # Pair p needs rows <= 8p+11 by ~(9.5 + 2.35p) us -- huge slack
        # after the first three chunks.
        rows = [20, 16, 20, 28, 36, 36, 41]
        assert sum(rows) * PW == FX
        bnd = [0]
        for nr in rows:
            bnd.append(bnd[-1] + nr * PW)
        nc.sync.dma_start(wst_s[:, 0:3 * DP], wst[:, 0:3 * DP])
        nc.sync.dma_start(XX[:, bnd[0]:bnd[1]], xx[:, bnd[0]:bnd[1]])
        nc.sync.dma_start(wst_s[:, 3 * DP:9 * DP], wst[:, 3 * DP:9 * DP])
        nc.sync.dma_start(XX[:, bnd[1]:bnd[2]], xx[:, bnd[1]:bnd[2]])
        nc.sync.dma_start(XX[:, bnd[2]:bnd[3]], xx[:, bnd[2]:bnd[3]])
        nc.sync.dma_start(wm_s[:], wm[:])
        for k in range(3, len(rows)):
            nc.sync.dma_start(XX[:, bnd[k]:bnd[k + 1]],
                              xx[:, bnd[k]:bnd[k + 1]])

        # base view and the (-1 col) view used by tile t1's (2,0) tap
        XV = XX[:, 0:F].rearrange("p (b h w) -> p b h w", b=BPC, h=PH, w=PW)
        XM = XX[:, PW - 1:PW - 1 + F].rearrange(
            "p (b h w) -> p b h w", b=BPC, h=PH, w=PW)

        # PE warm-up: dummy matmuls on a zeroed SBUF tile bridge the HAM
        # activity window CONTINUOUSLY from right after the preamble until
        # the first x chunk lands (~4.5us), so the clock-gate is at 8/8
        # when real matmuls start.  gpsimd memset: its queue is free
        # first.  A gap here restarts the 3.4us HAM ramp.
        warm = sb.tile([128, 256], bf16)
        nc.gpsimd.memset(warm[:], 0)
        wq = pcp.tile([128, 2 * PB], f32, name="pcq")
        for _ in range(15):
            nc.tensor.matmul(wq[:, 0:256], warm[:, 0:128], warm[:],
                             start=True, stop=True, skip_group_check=True)
        state = {"warmq": wq}

        def stage1_pair(b, t0):
            """Two tiles' conv groups: 9 single-tap K=64 matmuls each,
            t0 on PE rows 0:64 (plain x), t1 on rows 64:128 (diag-shifted
            x) -> the two tiles' taps run pairwise-concurrently."""
            t1 = t0 + 1
            h0, h1 = t0 * R, t1 * R
            pcq = state.pop("warmq", None)
            if pcq is None:
                pcq = pcp.tile([128, 2 * PB], f32, name="pcq")
            pcqv = pcq.rearrange("p (u n) -> p u n", u=2)
            pc0 = pcqv[:, 0, 0:N]
            pc1 = pcqv[:, 1, 0:N]
            for k, (kh, kw) in enumerate(TAPS):
                st, sp = k == 0, k == 8
                nc.tensor.matmul(pc0, wst_s[0:64, k * DP:(k + 1) * DP],
                                 XV[0:64, b, h0 + kh:h0 + kh + R, kw:kw + W],
                                 start=st, stop=sp)
                if kw == 0:
                    v1 = XM[64:128, b, h1 + kh - 2:h1 + kh - 2 + R, 0:W]
                else:
                    v1 = XV[64:128, b,
                            h1 + kh - 1:h1 + kh - 1 + R, kw - 1:kw - 1 + W]
                nc.tensor.matmul(pc1, wst_s[64:128, k * DP:(k + 1) * DP],
                                 v1, start=st, stop=sp)
            # one strided copy evacuates both tiles' conv PSUM banks; the
            # last pairs of batch 1 split across engines to shorten the
            # end-of-kernel dependency chain
            c1q = c1p.tile([128, 2 * N], bf16, name="c1q")
            c1qv = c1q.rearrange("p (u n) -> p u n", u=2)
            if b == BPC - 1 and t0 >= NT - 4:
                nc.vector.tensor_copy(c1qv[:, 0:1, :], pcqv[:, 0:1, 0:N])
                nc.scalar.copy(c1qv[:, 1:2, :], pcqv[:, 1:2, 0:N])
            elif (t0 // 2) % 2 == 0:
                nc.vector.tensor_copy(c1qv[:], pcqv[:, :, 0:N])
            else:
                nc.scalar.copy(c1qv[:], pcqv[:, :, 0:N])
            if STAGE2_SWAP:
                # partition-swapped copy of c1 so stage2's d-hi terms are
                # readable from PE rows 0:64 (and d-lo from rows 64:128):
                # keeps every PSUM bank single-row-group (cross-row-group
                # bank accumulation hard-faults the PE)
                c1s = c1sp.tile([128, 2 * N], bf16, name="c1s")
                nc.sync.dma_start(c1s[0:64, :], c1q[64:128, :])
                nc.sync.dma_start(c1s[64:128, :], c1q[0:64, :])
                state[("c1s", b, t0)] = c1s
            state[(b, t0)] = c1q[:, 0:N]
            state[(b, t1)] = c1q[:, N:2 * N]
            state[("c1q", b, t0)] = c1q

        def stage2_pair(b, t0):
            """[O,D] channel mix for tiles t0,t0+1 as four K=64 row-group
            slots over four PSUM banks.  Each bank is accumulated by two
            matmuls (o-half x d-lo on rows 0:64, then the same o-half x
            d-hi on rows 64:128) that are two slots apart, so no PSUM
            bank ever has two concurrent writers."""
            t1 = t0 + 1
            c1a = state.pop((b, t0))
            c1b = state.pop((b, t1))
            last = b == BPC - 1 and t0 >= NT - 4
            poa = pop.tile([128, 2 * PB], f32, name="po")
            pob = pop.tile([128, 2 * PB], f32, name="po")
            pva = poa.rearrange("p (u n) -> p u n", u=2)
            pvb = pob.rearrange("p (u n) -> p u n", u=2)
            if STAGE2_SWAP:
                # tile t0 entirely on PE rows 0:64 (banks in pva), t1 on
                # rows 64:128 (banks in pvb) -> pairwise concurrent, and
                # every bank is accumulated by two SAME-row-group matmuls
                # (d-lo via the native c1/wm, d-hi via the swapped pair)
                c1q = state.pop(("c1q", b, t0))
                c1s = state.pop(("c1s", b, t0))
                for u in (0, 1):                    # o-half
                    oc = u * 128
                    nc.tensor.matmul(pva[:, u, 0:N],
                                     wm_s[0:64, oc:oc + 128],
                                     c1q[0:64, 0:N], start=True, stop=False)
                    nc.tensor.matmul(pvb[:, u, 0:N],
                                     wm_s[64:128, oc:oc + 128],
                                     c1q[64:128, N:2 * N],
                                     start=True, stop=False)
                    nc.tensor.matmul(pva[:, u, 0:N],
                                     wm_s[0:64, O + oc:O + oc + 128],
                                     c1s[0:64, 0:N], start=False, stop=True)
                    nc.tensor.matmul(pvb[:, u, 0:N],
                                     wm_s[64:128, O + oc:O + oc + 128],
                                     c1s[64:128, N:2 * N],
                                     start=False, stop=True)
            else:
                for pv, c1 in ((pva, c1a), (pvb, c1b)):
                    nc.tensor.matmul(pv[:, 0, 0:N], wm_s[:, 0:128], c1,
                                     start=True, stop=True)
                    nc.tensor.matmul(pv[:, 1, 0:N], wm_s[:, 128:256], c1,
                                     start=True, stop=True)
            stg = stgp.tile([128, 4 * N], bf16, name="stg")
            stgv = stg.rearrange("p (u m) -> p u m", u=2)
            # strided copies evacuate both 128-channel halves; the very
            # last pair splits quarter-per-engine to shorten the tail
            if last:
                nc.scalar.copy(stgv[:, 0:1, 0:N], pva[:, 0:1, 0:N])
                nc.vector.tensor_copy(stgv[:, 1:2, 0:N], pva[:, 1:2, 0:N])
                nc.scalar.copy(stgv[:, 0:1, N:2 * N], pvb[:, 0:1, 0:N])
                nc.vector.tensor_copy(stgv[:, 1:2, N:2 * N],
                                      pvb[:, 1:2, 0:N])
            elif (t0 // 2) % 2 == 0:
                nc.scalar.copy(stgv[:, :, 0:N], pva[:, :, 0:N])
                nc.scalar.copy(stgv[:, :, N:2 * N], pvb[:, :, 0:N])
            else:
                nc.vector.tensor_copy(stgv[:, :, 0:N], pva[:, :, 0:N])
                nc.vector.tensor_copy(stgv[:, :, N:2 * N], pvb[:, :, 0:N])
            # output DMA: partition o carries channels {o, 128+o}; one
            # trigger per pair, except the final pair goes per-tile on
            # alternating rings so the drain overlaps
            ov = out[b].rearrange("(u o) h w -> o u (h w)", u=2)
            if last:
                # 4 quarter-DMAs on 3 rings so the final drain overlaps
                nc.gpsimd.dma_start(ov[:, 0:1, t0 * N:t1 * N],
                                    stgv[:, 0:1, 0:N])
                nc.scalar.dma_start(ov[:, 1:2, t0 * N:t1 * N],
                                    stgv[:, 1:2, 0:N])
                nc.sync.dma_start(ov[:, 0:1, t1 * N:(t1 + 1) * N],
                                  stgv[:, 0:1, N:2 * N])
                nc.scalar.dma_start(ov[:, 1:2, t1 * N:(t1 + 1) * N],
                                    stgv[:, 1:2, N:2 * N])
            else:
                dst = ov[:, :, t0 * N:(t1 + 1) * N]
                if b == 0 or STAGE2_SWAP:
                    nc.gpsimd.dma_start(dst, stgv)
                else:
                    nc.sync.dma_start(dst, stgv)

        NP = NT // 2
        PLAG = 3 if STAGE2_SWAP else 2   # stage2 lag in pairs
        # one continuous pipeline across the batch boundary: no flush
        # bubble between batches, only a single 2-pair flush at the end
        pairs = [(b, 2 * p) for b in range(BPC) for p in range(NP)]
        for g, bt in enumerate(pairs):
            stage1_pair(*bt)
            if g >= PLAG:
                stage2_pair(*pairs[g - PLAG])
        for g in range(len(pairs) - PLAG, len(pairs)):
            stage2_pair(*pairs[g])

    nc.compile()
    return nc


def _get_nc():
    if "nc" not in _NC_CACHE:
        _NC_CACHE["nc"] = _build()
    return _NC_CACHE["nc"]


def _prep_inputs(x, dictionary, lookup_coefficients, lookup_indices):
    x = np.asarray(x, dtype=np.float32)
    dic = np.asarray(dictionary, dtype=np.float32)
    coeff = np.asarray(lookup_coefficients, dtype=np.float32).reshape(O, -1)
    idx = np.asarray(lookup_indices).astype(np.int64).reshape(O, -1)

    wmat = np.zeros((O, D), np.float32)
    np.add.at(wmat, (np.arange(O)[:, None], idx), coeff)
    wmp = np.zeros((DP, O), np.float32)
    wmp[:D] = wmat.T
    # block 2 = rows rolled by 64: row r holds Wmat.T[(r+64)%128], read
    # against the partition-swapped c1 copy in stage 2
    wmp = np.concatenate([wmp, np.roll(wmp, -64, axis=0)],
                         axis=1).astype(ml_dtypes.bfloat16)

    # stationary slabs [128, 9*DP]: one slab per tap, duplicated into
    # both row halves for the K=64 row-group matmuls
    dt_ = dic.transpose(1, 0, 2, 3)                       # [cin, d, kh, kw]
    wstk = np.zeros((128, 9 * DP), np.float32)
    for k, (kh, kw) in enumerate(TAPS):
        wstk[0:64, k * DP:k * DP + D] = dt_[:, :, kh, kw]
        wstk[64:128, k * DP:k * DP + D] = dt_[:, :, kh, kw]
    wstk = wstk.astype(ml_dtypes.bfloat16)

    xpad = np.zeros((B, CIN, PH, PW), np.float32)
    xpad[:, :, 1:H + 1, 1:W + 1] = x
    xpad = xpad.astype(ml_dtypes.bfloat16)

    in_maps = []
    for c in range(NCORES):
        xf = xpad[c * BPC:(c + 1) * BPC].transpose(1, 0, 2, 3).reshape(CIN, F)
        xxk = np.zeros((128, FX), ml_dtypes.bfloat16)
        xxk[0:64, 0:F] = xf
        xxk[64:128, 0:F - PW - 1] = xf[:, PW + 1:]     # (+1 row, +1 col)
        in_maps.append({
            "xx": np.ascontiguousarray(xxk),
            "wst": wstk, "wm": wmp,
        })
    return in_maps


def _run(in_maps, trace=False, **kw):
    nc = _get_nc()
    return run_bass_kernel_spmd(nc, in_maps, core_ids=list(range(NCORES)),
                                trace=trace, **kw)


def kernel(x, dictionary, lookup_coefficients, lookup_indices):
    in_maps = _prep_inputs(x, dictionary, lookup_coefficients, lookup_indices)
    res = _run(in_maps)
    outs = [np.asarray(res.results[c]["out"]).astype(np.float32)
            for c in range(NCORES)]
    return np.concatenate(outs, axis=0)


# revision 34
# speedup vs baseline: 1.2034x; 1.2034x over previous
"""LCNN conv2d kernel for Trainium2 (8 NeuronCores, batch-sharded).

Math: out[b,o,h,w] = sum_d Wmat[o,d] * conv2d(x, dictionary)[b,d,h,w]
where Wmat is the scatter-add of lookup_coefficients into [O, D].

Device strategy (per core, 2 batches), all-bf16, all matmuls K=64
row-group tiled so pairs run concurrently in the PE array and there are
no full-array<->row-group transitions (each costs ~250 ns):
 - input buffer XX [128, F+PW]: rows 0:64 = padded x, rows 64:128 = x
   shifted by (1 row + 1 col).  Tile t0 of a pair computes its 9 conv
   taps on PE rows 0:64 (plain view offsets), tile t1 concurrently on
   rows 64:128 (offsets shifted (-1,-1); the (2,0) tap uses a -1-column
   view of the same buffer).  4.5 effective PE slots per tile, half the
   input HBM traffic of 4-way duplication (4.9 MB vs 9.8 MB per core).
 - stage 2 [O=256, D->128] also K=64-split: two slots per tile, each
   running (o-half, d-lo) on rows 0:64 concurrently with the other
   o-half x d-hi on rows 64:128, cross-scheduled over two PSUM banks so
   no bank is written by two matmuls at once.
 - outputs staged as bf16, DMA'd once per tile-pair (b=0 pairs on the
   gpsimd SWDGE ring, b=1 on the sync HWDGE ring); weights go on the
   scalar HWDGE ring so they land in parallel with the first x chunks;
   host upcasts to f32.
"""
import os
import sys

for _p in ("/opt/trn_rl_repo", "/root/.axon_site/_ro/trn_rl_repo"):
    if os.path.isdir(_p) and _p not in sys.path:
        sys.path.insert(0, _p)

import ml_dtypes
import numpy as np
from contextlib import ExitStack

from concourse import bacc, mybir, tile
from concourse.bass_utils import run_bass_kernel_spmd

# problem shapes (hardcoded per contract)
B, CIN, H, W = 16, 64, 96, 96
D, O = 100, 256
DP = 128                   # D padded to full PE width
NCORES = 8
BPC = B // NCORES          # batches per core
PH, PW = H + 2, W + 2      # zero-padded spatial
F = BPC * PH * PW          # per-partition x extent
FX = F + PW                # + tail pad so the (-1 col) view stays in-bounds
R = 4                      # output rows per matmul tile
NT = H // R                # h-tiles per batch
N = R * W                  # matmul free size (384)
PB = 512                   # psum bank stride (f32 elems)
TAPS = [(kh, kw) for kh in range(3) for kw in range(3)]
STAGE2_SWAP = False        # row-group stage2 via c1 partition-swap DMA:
                           # correct but the per-pair swap chain stalls
                           # stage2 ~560ns/pair even with a dedicated
                           # ring; full-array stage2 measured faster
WEIGHTS_RING = "scalar"    # which engine ring loads wst/wm
bf16 = mybir.dt.bfloat16
f32 = mybir.dt.float32

_NC_CACHE = {}


def _build():
    nc = bacc.Bacc(None, target_bir_lowering=False, debug=False)
    xx = nc.declare_dram_parameter("xx", [128, FX], bf16, isOutput=False)
    wst = nc.declare_dram_parameter("wst", [128, 9 * DP], bf16, isOutput=False)
    wm = nc.declare_dram_parameter("wm", [DP, 2 * O], bf16, isOutput=False)
    out = nc.declare_dram_parameter("out", [BPC, O, H, W], bf16, isOutput=True)

    with tile.TileContext(nc) as tc, ExitStack() as ctx:
        sb = ctx.enter_context(tc.tile_pool(name="sb", bufs=1))
        c1p = ctx.enter_context(tc.tile_pool(name="c1p", bufs=6))
        c1sp = ctx.enter_context(tc.tile_pool(name="c1sp", bufs=6))
        stgp = ctx.enter_context(tc.tile_pool(name="stgp", bufs=6))
        pcp = ctx.enter_context(tc.tile_pool(name="pcp", bufs=2, space="PSUM"))
        pop = ctx.enter_context(tc.tile_pool(name="pop", bufs=2, space="PSUM"))

        XX = sb.tile([128, FX], bf16)
        wst_s = sb.tile([128, 9 * DP], bf16)
        wm_s = sb.tile([DP, 2 * O], bf16)
        # weights via the scalar HWDGE ring: lands in parallel with the
        # sync ring's first x chunks, well before the first real matmul
        # startup is gated by CUMULATIVE early DMA bytes (shared SDMA
        # bandwidth), so order the sync ring by first-use: tap 0-2 slabs,
        # the 12 rows pair 0 reads, the remaining slabs, then the rest.
        # Pair p needs rows <= 8p+11 by ~(9.5 + 2.35p) us -- huge slack
        # after the first three chunks.
        rows = [20, 16, 20, 28, 36, 36, 41]
        assert sum(rows) * PW == FX
        bnd = [0]
        for nr in rows:
            bnd.append(bnd[-1] + nr * PW)
        nc.sync.dma_start(wst_s[:, 0:3 * DP], wst[:, 0:3 * DP])
        nc.sync.dma_start(XX[:, bnd[0]:bnd[1]], xx[:, bnd[0]:bnd[1]])
        nc.sync.dma_start(wst_s[:, 3 * DP:9 * DP], wst[:, 3 * DP:9 * DP])
        nc.sync.dma_start(XX[:, bnd[1]:bnd[2]], xx[:, bnd[1]:bnd[2]])
        nc.sync.dma_start(XX[:, bnd[2]:bnd[3]], xx[:, bnd[2]:bnd[3]])
        nc.sync.dma_start(wm_s[:], wm[:])
        for k in range(3, len(rows)):
            nc.sync.dma_start(XX[:, bnd[k]:bnd[k + 1]],
                              xx[:, bnd[k]:bnd[k + 1]])

        # base view and the (-1 col) view used by tile t1's (2,0) tap
        XV = XX[:, 0:F].rearrange("p (b h w) -> p b h w", b=BPC, h=PH, w=PW)
        XM = XX[:, PW - 1:PW - 1 + F].rearrange(
            "p (b h w) -> p b h w", b=BPC, h=PH, w=PW)

        # PE warm-up: dummy matmuls on a zeroed SBUF tile bridge the HAM
        # activity window CONTINUOUSLY from right after the preamble until
        # the first x chunk lands (~4.5us), so the clock-gate is at 8/8
        # when real matmuls start.  gpsimd memset: its queue is free
        # first.  A gap here restarts the 3.4us HAM ramp.
        warm = sb.tile([128, 256], bf16)
        nc.gpsimd.memset(warm[:], 0)
        wq = pcp.tile([128, 2 * PB], f32, name="pcq")
        for _ in range(15):
            nc.tensor.matmul(wq[:, 0:256], warm[:, 0:128], warm[:],
                             start=True, stop=True, skip_group_check=True)
        state = {"warmq": wq}

        def stage1_pair(b, t0):
            """Two tiles' conv groups: 9 single-tap K=64 matmuls each,
            t0 on PE rows 0:64 (plain x), t1 on rows 64:128 (diag-shifted
            x) -> the two tiles' taps run pairwise-concurrently."""
            t1 = t0 + 1
            h0, h1 = t0 * R, t1 * R
            pcq = state.pop("warmq", None)
            if pcq is None:
                pcq = pcp.tile([128, 2 * PB], f32, name="pcq")
            pcqv = pcq.rearrange("p (u n) -> p u n", u=2)
            pc0 = pcqv[:, 0, 0:N]
            pc1 = pcqv[:, 1, 0:N]
            for k, (kh, kw) in enumerate(TAPS):
                st, sp = k == 0, k == 8
                nc.tensor.matmul(pc0, wst_s[0:64, k * DP:(k + 1) * DP],
                                 XV[0:64, b, h0 + kh:h0 + kh + R, kw:kw + W],
                                 start=st, stop=sp)
                if kw == 0:
                    v1 = XM[64:128, b, h1 + kh - 2:h1 + kh - 2 + R, 0:W]
                else:
                    v1 = XV[64:128, b,
                            h1 + kh - 1:h1 + kh - 1 + R, kw - 1:kw - 1 + W]
                nc.tensor.matmul(pc1, wst_s[64:128, k * DP:(k + 1) * DP],
                                 v1, start=st, stop=sp)
            # one strided copy evacuates both tiles' conv PSUM banks; the
            # last pairs of batch 1 split across engines to shorten the
            # end-of-kernel dependency chain
            c1q = c1p.tile([128, 2 * N], bf16, name="c1q")
            c1qv = c1q.rearrange("p (u n) -> p u n", u=2)
            if b == BPC - 1 and t0 >= NT - 4:
                nc.vector.tensor_copy(c1qv[:, 0:1, :], pcqv[:, 0:1, 0:N])
                nc.scalar.copy(c1qv[:, 1:2, :], pcqv[:, 1:2, 0:N])
            elif (t0 // 2) % 2 == 0:
                nc.vector.tensor_copy(c1qv[:], pcqv[:, :, 0:N])
            else:
                nc.scalar.copy(c1qv[:], pcqv[:, :, 0:N])
            if STAGE2_SWAP:
                # partition-swapped copy of c1 so stage2's d-hi terms are
                # readable from PE rows 0:64 (and d-lo from rows 64:128):
                # keeps every PSUM bank single-row-group (cross-row-group
                # bank accumulation hard-faults the PE)
                c1s = c1sp.tile([128, 2 * N], bf16, name="c1s")
                nc.sync.dma_start(c1s[0:64, :], c1q[64:128, :])
                nc.sync.dma_start(c1s[64:128, :], c1q[0:64, :])
                state[("c1s", b, t0)] = c1s
            state[(b, t0)] = c1q[:, 0:N]
            state[(b, t1)] = c1q[:, N:2 * N]
            state[("c1q", b, t0)] = c1q

        def stage2_pair(b, t0):
            """[O,D] channel mix for tiles t0,t0+1 as four K=64 row-group
            slots over four PSUM banks.  Each bank is accumulated by two
            matmuls (o-half x d-lo on rows 0:64, then the same o-half x
            d-hi on rows 64:128) that are two slots apart, so no PSUM
            bank ever has two concurrent writers."""
            t1 = t0 + 1
            c1a = state.pop((b, t0))
            c1b = state.pop((b, t1))
            last = b == BPC - 1 and t0 >= NT - 4
            poa = pop.tile([128, 2 * PB], f32, name="po")
            pob = pop.tile([128, 2 * PB], f32, name="po")
            pva = poa.rearrange("p (u n) -> p u n", u=2)
            pvb = pob.rearrange("p (u n) -> p u n", u=2)
            if STAGE2_SWAP:
                # tile t0 entirely on PE rows 0:64 (banks in pva), t1 on
                # rows 64:128 (banks in pvb) -> pairwise concurrent, and
                # every bank is accumulated by two SAME-row-group matmuls
                # (d-lo via the native c1/wm, d-hi via the swapped pair)
                c1q = state.pop(("c1q", b, t0))
                c1s = state.pop(("c1s", b, t0))
                for u in (0, 1):                    # o-half
                    oc = u * 128
                    nc.tensor.matmul(pva[:, u, 0:N],
                                     wm_s[0:64, oc:oc + 128],
                                     c1q[0:64, 0:N], start=True, stop=False)
                    nc.tensor.matmul(pvb[:, u, 0:N],
                                     wm_s[64:128, oc:oc + 128],
                                     c1q[64:128, N:2 * N],
                                     start=True, stop=False)
                    nc.tensor.matmul(pva[:, u, 0:N],
                                     wm_s[0:64, O + oc:O + oc + 128],
                                     c1s[0:64, 0:N], start=False, stop=True)
                    nc.tensor.matmul(pvb[:, u, 0:N],
                                     wm_s[64:128, O + oc:O + oc + 128],
                                     c1s[64:128, N:2 * N],
                                     start=False, stop=True)
            else:
                for pv, c1 in ((pva, c1a), (pvb, c1b)):
                    nc.tensor.matmul(pv[:, 0, 0:N], wm_s[:, 0:128], c1,
                                     start=True, stop=True)
                    nc.tensor.matmul(pv[:, 1, 0:N], wm_s[:, 128:256], c1,
                                     start=True, stop=True)
            stg = stgp.tile([128, 4 * N], bf16, name="stg")
            stgv = stg.rearrange("p (u m) -> p u m", u=2)
            # strided copies evacuate both 128-channel halves; the very
            # last pair splits quarter-per-engine to shorten the tail
            if last:
                nc.scalar.copy(stgv[:, 0:1, 0:N], pva[:, 0:1, 0:N])
                nc.vector.tensor_copy(stgv[:, 1:2, 0:N], pva[:, 1:2, 0:N])
                nc.scalar.copy(stgv[:, 0:1, N:2 * N], pvb[:, 0:1, 0:N])
                nc.vector.tensor_copy(stgv[:, 1:2, N:2 * N],
                                      pvb[:, 1:2, 0:N])
            elif (t0 // 2) % 2 == 0:
                nc.scalar.copy(stgv[:, :, 0:N], pva[:, :, 0:N])
                nc.scalar.copy(stgv[:, :, N:2 * N], pvb[:, :, 0:N])
            else:
                nc.vector.tensor_copy(stgv[:, :, 0:N], pva[:, :, 0:N])
                nc.vector.tensor_copy(stgv[:, :, N:2 * N], pvb[:, :, 0:N])
            # output DMA: partition o carries channels {o, 128+o}; one
            # trigger per pair, except the final pair goes per-tile on
            # alternating rings so the drain overlaps
            ov = out[b].rearrange("(u o) h w -> o u (h w)", u=2)
            if last:
                # 4 quarter-DMAs on 3 rings so the final drain overlaps
                nc.gpsimd.dma_start(ov[:, 0:1, t0 * N:t1 * N],
                                    stgv[:, 0:1, 0:N])
                nc.scalar.dma_start(ov[:, 1:2, t0 * N:t1 * N],
                                    stgv[:, 1:2, 0:N])
                nc.sync.dma_start(ov[:, 0:1, t1 * N:(t1 + 1) * N],
                                  stgv[:, 0:1, N:2 * N])
                nc.scalar.dma_start(ov[:, 1:2, t1 * N:(t1 + 1) * N],
                                    stgv[:, 1:2, N:2 * N])
            else:
                dst = ov[:, :, t0 * N:(t1 + 1) * N]
                if b == 0 or STAGE2_SWAP:
                    nc.gpsimd.dma_start(dst, stgv)
                else:
                    nc.sync.dma_start(dst, stgv)

        NP = NT // 2
        PLAG = 3 if STAGE2_SWAP else 2   # stage2 lag in pairs
        # one continuous pipeline across the batch boundary: no flush
        # bubble between batches, only a single 2-pair flush at the end
        pairs = [(b, 2 * p) for b in range(BPC) for p in range(NP)]
        for g, bt in enumerate(pairs):
            stage1_pair(*bt)
            if g >= PLAG:
                stage2_pair(*pairs[g - PLAG])
        for g in range(len(pairs) - PLAG, len(pairs)):
            stage2_pair(*pairs[g])

    nc.compile()
    return nc


def _get_nc():
    if "nc" not in _NC_CACHE:
        _NC_CACHE["nc"] = _build()
    return _NC_CACHE["nc"]


def _prep_inputs(x, dictionary, lookup_coefficients, lookup_indices):
    x = np.asarray(x, dtype=np.float32)
    dic = np.asarray(dictionary, dtype=np.float32)
    coeff = np.asarray(lookup_coefficients, dtype=np.float32).reshape(O, -1)
    idx = np.asarray(lookup_indices).astype(np.int64).reshape(O, -1)

    wmat = np.zeros((O, D), np.float32)
    np.add.at(wmat, (np.arange(O)[:, None], idx), coeff)
    wmp = np.zeros((DP, O), np.float32)
    wmp[:D] = wmat.T
    # block 2 = rows rolled by 64: row r holds Wmat.T[(r+64)%128], read
    # against the partition-swapped c1 copy in stage 2
    wmp = np.concatenate([wmp, np.roll(wmp, -64, axis=0)],
                         axis=1).astype(ml_dtypes.bfloat16)

    # stationary slabs [128, 9*DP]: one slab per tap, duplicated into
    # both row halves for the K=64 row-group matmuls
    dt_ = dic.transpose(1, 0, 2, 3)                       # [cin, d, kh, kw]
    wstk = np.zeros((128, 9 * DP), np.float32)
    for k, (kh, kw) in enumerate(TAPS):
        wstk[0:64, k * DP:k * DP + D] = dt_[:, :, kh, kw]
        wstk[64:128, k * DP:k * DP + D] = dt_[:, :, kh, kw]
    wstk = wstk.astype(ml_dtypes.bfloat16)

    xpad = np.zeros((B, CIN, PH, PW), np.float32)
    xpad[:, :, 1:H + 1, 1:W + 1] = x
    xpad = xpad.astype(ml_dtypes.bfloat16)

    in_maps = []
    for c in range(NCORES):
        xf = xpad[c * BPC:(c + 1) * BPC].transpose(1, 0, 2, 3).reshape(CIN, F)
        xxk = np.zeros((128, FX), ml_dtypes.bfloat16)
        xxk[0:64, 0:F] = xf
        xxk[64:128, 0:F - PW - 1] = xf[:, PW + 1:]     # (+1 row, +1 col)
        in_maps.append({
            "xx": np.ascontiguousarray(xxk),
            "wst": wstk, "wm": wmp,
        })
    return in_maps


def _run(in_maps, trace=False, **kw):
    nc = _get_nc()
    return run_bass_kernel_spmd(nc, in_maps, core_ids=list(range(NCORES)),
                                trace=trace, **kw)


def kernel(x, dictionary, lookup_coefficients, lookup_indices):
    in_maps = _prep_inputs(x, dictionary, lookup_coefficients, lookup_indices)
    res = _run(in_maps)
    outs = [np.asarray(res.results[c]["out"]).astype(np.float32)
            for c in range(NCORES)]
    return np.concatenate(outs, axis=0)


# revision 35
# speedup vs baseline: 1.2194x; 1.0133x over previous
"""LCNN conv2d kernel for Trainium2 (8 NeuronCores, batch-sharded).

Math: out[b,o,h,w] = sum_d Wmat[o,d] * conv2d(x, dictionary)[b,d,h,w]
where Wmat is the scatter-add of lookup_coefficients into [O, D].

Device strategy (per core, 2 batches), all-bf16, all matmuls K=64
row-group tiled so pairs run concurrently in the PE array and there are
no full-array<->row-group transitions (each costs ~250 ns):
 - input buffer XX [128, F+PW]: rows 0:64 = padded x, rows 64:128 = x
   shifted by (1 row + 1 col).  Tile t0 of a pair computes its 9 conv
   taps on PE rows 0:64 (plain view offsets), tile t1 concurrently on
   rows 64:128 (offsets shifted (-1,-1); the (2,0) tap uses a -1-column
   view of the same buffer).  4.5 effective PE slots per tile, half the
   input HBM traffic of 4-way duplication (4.9 MB vs 9.8 MB per core).
 - stage 2 [O=256, D->128] also K=64-split: two slots per tile, each
   running (o-half, d-lo) on rows 0:64 concurrently with the other
   o-half x d-hi on rows 64:128, cross-scheduled over two PSUM banks so
   no bank is written by two matmuls at once.
 - outputs staged as bf16, DMA'd once per tile-pair (b=0 pairs on the
   gpsimd SWDGE ring, b=1 on the sync HWDGE ring); weights go on the
   scalar HWDGE ring so they land in parallel with the first x chunks;
   host upcasts to f32.
"""
import os
import sys

for _p in ("/opt/trn_rl_repo", "/root/.axon_site/_ro/trn_rl_repo"):
    if os.path.isdir(_p) and _p not in sys.path:
        sys.path.insert(0, _p)

import ml_dtypes
import numpy as np
from contextlib import ExitStack

from concourse import bacc, mybir, tile
from concourse.bass_utils import run_bass_kernel_spmd

# problem shapes (hardcoded per contract)
B, CIN, H, W = 16, 64, 96, 96
D, O = 100, 256
DP = 128                   # D padded to full PE width
NCORES = 8
BPC = B // NCORES          # batches per core
PH, PW = H + 2, W + 2      # zero-padded spatial
F = BPC * PH * PW          # per-partition x extent
FX = F + PW                # + tail pad so the (-1 col) view stays in-bounds
R = 4                      # output rows per matmul tile
NT = H // R                # h-tiles per batch
N = R * W                  # matmul free size (384)
PB = 512                   # psum bank stride (f32 elems)
TAPS = [(kh, kw) for kh in range(3) for kw in range(3)]
STAGE2_SWAP = False        # row-group stage2 via c1 partition-swap DMA:
                           # correct but the per-pair swap chain stalls
                           # stage2 ~560ns/pair even with a dedicated
                           # ring; full-array stage2 measured faster
WEIGHTS_RING = "scalar"    # which engine ring loads wst/wm
bf16 = mybir.dt.bfloat16
f32 = mybir.dt.float32

_NC_CACHE = {}


def _build():
    nc = bacc.Bacc(None, target_bir_lowering=False, debug=False)
    xx = nc.declare_dram_parameter("xx", [128, FX], bf16, isOutput=False)
    wst = nc.declare_dram_parameter("wst", [128, 9 * DP], bf16, isOutput=False)
    wm = nc.declare_dram_parameter("wm", [DP, 2 * O], bf16, isOutput=False)
    out = nc.declare_dram_parameter("out", [BPC, O, H, W], bf16, isOutput=True)

    with tile.TileContext(nc) as tc, ExitStack() as ctx:
        sb = ctx.enter_context(tc.tile_pool(name="sb", bufs=1))
        c1p = ctx.enter_context(tc.tile_pool(name="c1p", bufs=6))
        c1sp = ctx.enter_context(tc.tile_pool(name="c1sp", bufs=6))
        stgp = ctx.enter_context(tc.tile_pool(name="stgp", bufs=6))
        pcp = ctx.enter_context(tc.tile_pool(name="pcp", bufs=2, space="PSUM"))
        pop = ctx.enter_context(tc.tile_pool(name="pop", bufs=2, space="PSUM"))

        XX = sb.tile([128, FX], bf16)
        wst_s = sb.tile([128, 9 * DP], bf16)
        wm_s = sb.tile([DP, 2 * O], bf16)
        # weights via the scalar HWDGE ring: lands in parallel with the
        # sync ring's first x chunks, well before the first real matmul
        # startup is gated by CUMULATIVE early DMA bytes (shared SDMA
        # bandwidth), so order the sync ring by first-use: tap 0-2 slabs,
        # the 12 rows pair 0 reads, the remaining slabs, then the rest.
        # Pair p needs rows <= 8p+11 by ~(9.5 + 2.35p) us -- huge slack
        # after the first three chunks.
        rows = [20, 16, 20, 28, 36, 36, 41]
        assert sum(rows) * PW == FX
        bnd = [0]
        for nr in rows:
            bnd.append(bnd[-1] + nr * PW)
        nc.sync.dma_start(wst_s[:, 0:3 * DP], wst[:, 0:3 * DP])
        nc.sync.dma_start(XX[:, bnd[0]:bnd[1]], xx[:, bnd[0]:bnd[1]])
        nc.sync.dma_start(wst_s[:, 3 * DP:9 * DP], wst[:, 3 * DP:9 * DP])
        nc.sync.dma_start(XX[:, bnd[1]:bnd[2]], xx[:, bnd[1]:bnd[2]])
        nc.sync.dma_start(XX[:, bnd[2]:bnd[3]], xx[:, bnd[2]:bnd[3]])
        nc.sync.dma_start(wm_s[:], wm[:])
        for k in range(3, len(rows)):
            nc.sync.dma_start(XX[:, bnd[k]:bnd[k + 1]],
                              xx[:, bnd[k]:bnd[k + 1]])

        # base view and the (-1 col) view used by tile t1's (2,0) tap
        XV = XX[:, 0:F].rearrange("p (b h w) -> p b h w", b=BPC, h=PH, w=PW)
        XM = XX[:, PW - 1:PW - 1 + F].rearrange(
            "p (b h w) -> p b h w", b=BPC, h=PH, w=PW)

        # PE warm-up: dummy matmuls on a zeroed SBUF tile bridge the HAM
        # activity window CONTINUOUSLY from right after the preamble until
        # the first x chunk lands (~4.5us), so the clock-gate is at 8/8
        # when real matmuls start.  gpsimd memset: its queue is free
        # first.  A gap here restarts the 3.4us HAM ramp.
        warm = sb.tile([128, 256], bf16)
        nc.gpsimd.memset(warm[:], 0)
        wq = pcp.tile([128, 2 * PB], f32, name="pcq")
        for _ in range(19):
            nc.tensor.matmul(wq[:, 0:256], warm[:, 0:128], warm[:],
                             start=True, stop=True, skip_group_check=True)
        state = {"warmq": wq}

        def stage1_pair(b, t0):
            """Two tiles' conv groups: 9 single-tap K=64 matmuls each,
            t0 on PE rows 0:64 (plain x), t1 on rows 64:128 (diag-shifted
            x) -> the two tiles' taps run pairwise-concurrently."""
            t1 = t0 + 1
            h0, h1 = t0 * R, t1 * R
            pcq = state.pop("warmq", None)
            if pcq is None:
                pcq = pcp.tile([128, 2 * PB], f32, name="pcq")
            pcqv = pcq.rearrange("p (u n) -> p u n", u=2)
            pc0 = pcqv[:, 0, 0:N]
            pc1 = pcqv[:, 1, 0:N]
            for k, (kh, kw) in enumerate(TAPS):
                st, sp = k == 0, k == 8
                nc.tensor.matmul(pc0, wst_s[0:64, k * DP:(k + 1) * DP],
                                 XV[0:64, b, h0 + kh:h0 + kh + R, kw:kw + W],
                                 start=st, stop=sp)
                if kw == 0:
                    v1 = XM[64:128, b, h1 + kh - 2:h1 + kh - 2 + R, 0:W]
                else:
                    v1 = XV[64:128, b,
                            h1 + kh - 1:h1 + kh - 1 + R, kw - 1:kw - 1 + W]
                nc.tensor.matmul(pc1, wst_s[64:128, k * DP:(k + 1) * DP],
                                 v1, start=st, stop=sp)
            # one strided copy evacuates both tiles' conv PSUM banks; the
            # last pairs of batch 1 split across engines to shorten the
            # end-of-kernel dependency chain
            c1q = c1p.tile([128, 2 * N], bf16, name="c1q")
            c1qv = c1q.rearrange("p (u n) -> p u n", u=2)
            if b == BPC - 1 and t0 >= NT - 4:
                nc.vector.tensor_copy(c1qv[:, 0:1, :], pcqv[:, 0:1, 0:N])
                nc.scalar.copy(c1qv[:, 1:2, :], pcqv[:, 1:2, 0:N])
            elif (t0 // 2) % 2 == 0:
                nc.vector.tensor_copy(c1qv[:], pcqv[:, :, 0:N])
            else:
                nc.scalar.copy(c1qv[:], pcqv[:, :, 0:N])
            if STAGE2_SWAP:
                # partition-swapped copy of c1 so stage2's d-hi terms are
                # readable from PE rows 0:64 (and d-lo from rows 64:128):
                # keeps every PSUM bank single-row-group (cross-row-group
                # bank accumulation hard-faults the PE)
                c1s = c1sp.tile([128, 2 * N], bf16, name="c1s")
                nc.sync.dma_start(c1s[0:64, :], c1q[64:128, :])
                nc.sync.dma_start(c1s[64:128, :], c1q[0:64, :])
                state[("c1s", b, t0)] = c1s
            state[(b, t0)] = c1q[:, 0:N]
            state[(b, t1)] = c1q[:, N:2 * N]
            state[("c1q", b, t0)] = c1q

        def stage2_pair(b, t0):
            """[O,D] channel mix for tiles t0,t0+1 as four K=64 row-group
            slots over four PSUM banks.  Each bank is accumulated by two
            matmuls (o-half x d-lo on rows 0:64, then the same o-half x
            d-hi on rows 64:128) that are two slots apart, so no PSUM
            bank ever has two concurrent writers."""
            t1 = t0 + 1
            c1a = state.pop((b, t0))
            c1b = state.pop((b, t1))
            last = b == BPC - 1 and t0 >= NT - 4
            poa = pop.tile([128, 2 * PB], f32, name="po")
            pob = pop.tile([128, 2 * PB], f32, name="po")
            pva = poa.rearrange("p (u n) -> p u n", u=2)
            pvb = pob.rearrange("p (u n) -> p u n", u=2)
            if STAGE2_SWAP:
                # tile t0 entirely on PE rows 0:64 (banks in pva), t1 on
                # rows 64:128 (banks in pvb) -> pairwise concurrent, and
                # every bank is accumulated by two SAME-row-group matmuls
                # (d-lo via the native c1/wm, d-hi via the swapped pair)
                c1q = state.pop(("c1q", b, t0))
                c1s = state.pop(("c1s", b, t0))
                for u in (0, 1):                    # o-half
                    oc = u * 128
                    nc.tensor.matmul(pva[:, u, 0:N],
                                     wm_s[0:64, oc:oc + 128],
                                     c1q[0:64, 0:N], start=True, stop=False)
                    nc.tensor.matmul(pvb[:, u, 0:N],
                                     wm_s[64:128, oc:oc + 128],
                                     c1q[64:128, N:2 * N],
                                     start=True, stop=False)
                    nc.tensor.matmul(pva[:, u, 0:N],
                                     wm_s[0:64, O + oc:O + oc + 128],
                                     c1s[0:64, 0:N], start=False, stop=True)
                    nc.tensor.matmul(pvb[:, u, 0:N],
                                     wm_s[64:128, O + oc:O + oc + 128],
                                     c1s[64:128, N:2 * N],
                                     start=False, stop=True)
            else:
                for pv, c1 in ((pva, c1a), (pvb, c1b)):
                    nc.tensor.matmul(pv[:, 0, 0:N], wm_s[:, 0:128], c1,
                                     start=True, stop=True)
                    nc.tensor.matmul(pv[:, 1, 0:N], wm_s[:, 128:256], c1,
                                     start=True, stop=True)
            stg = stgp.tile([128, 4 * N], bf16, name="stg")
            stgv = stg.rearrange("p (u m) -> p u m", u=2)
            # strided copies evacuate both 128-channel halves; the very
            # last pair splits quarter-per-engine to shorten the tail
            if last:
                nc.scalar.copy(stgv[:, 0:1, 0:N], pva[:, 0:1, 0:N])
                nc.vector.tensor_copy(stgv[:, 1:2, 0:N], pva[:, 1:2, 0:N])
                nc.scalar.copy(stgv[:, 0:1, N:2 * N], pvb[:, 0:1, 0:N])
                nc.vector.tensor_copy(stgv[:, 1:2, N:2 * N],
                                      pvb[:, 1:2, 0:N])
            elif (t0 // 2) % 2 == 0:
                nc.scalar.copy(stgv[:, :, 0:N], pva[:, :, 0:N])
                nc.scalar.copy(stgv[:, :, N:2 * N], pvb[:, :, 0:N])
            else:
                nc.vector.tensor_copy(stgv[:, :, 0:N], pva[:, :, 0:N])
                nc.vector.tensor_copy(stgv[:, :, N:2 * N], pvb[:, :, 0:N])
            # output DMA: partition o carries channels {o, 128+o}; one
            # trigger per pair, except the final pair goes per-tile on
            # alternating rings so the drain overlaps
            ov = out[b].rearrange("(u o) h w -> o u (h w)", u=2)
            if last:
                # 4 quarter-DMAs on 3 rings so the final drain overlaps
                nc.gpsimd.dma_start(ov[:, 0:1, t0 * N:t1 * N],
                                    stgv[:, 0:1, 0:N])
                nc.scalar.dma_start(ov[:, 1:2, t0 * N:t1 * N],
                                    stgv[:, 1:2, 0:N])
                nc.sync.dma_start(ov[:, 0:1, t1 * N:(t1 + 1) * N],
                                  stgv[:, 0:1, N:2 * N])
                nc.scalar.dma_start(ov[:, 1:2, t1 * N:(t1 + 1) * N],
                                    stgv[:, 1:2, N:2 * N])
            else:
                dst = ov[:, :, t0 * N:(t1 + 1) * N]
                if b == 0 or STAGE2_SWAP:
                    nc.gpsimd.dma_start(dst, stgv)
                else:
                    nc.sync.dma_start(dst, stgv)

        NP = NT // 2
        PLAG = 3 if STAGE2_SWAP else 2   # stage2 lag in pairs
        # one continuous pipeline across the batch boundary: no flush
        # bubble between batches, only a single 2-pair flush at the end
        pairs = [(b, 2 * p) for b in range(BPC) for p in range(NP)]
        for g, bt in enumerate(pairs):
            stage1_pair(*bt)
            if g >= PLAG:
                stage2_pair(*pairs[g - PLAG])
        for g in range(len(pairs) - PLAG, len(pairs)):
            stage2_pair(*pairs[g])

    nc.compile()
    return nc


def _get_nc():
    if "nc" not in _NC_CACHE:
        _NC_CACHE["nc"] = _build()
    return _NC_CACHE["nc"]


def _prep_inputs(x, dictionary, lookup_coefficients, lookup_indices):
    x = np.asarray(x, dtype=np.float32)
    dic = np.asarray(dictionary, dtype=np.float32)
    coeff = np.asarray(lookup_coefficients, dtype=np.float32).reshape(O, -1)
    idx = np.asarray(lookup_indices).astype(np.int64).reshape(O, -1)

    wmat = np.zeros((O, D), np.float32)
    np.add.at(wmat, (np.arange(O)[:, None], idx), coeff)
    wmp = np.zeros((DP, O), np.float32)
    wmp[:D] = wmat.T
    # block 2 = rows rolled by 64: row r holds Wmat.T[(r+64)%128], read
    # against the partition-swapped c1 copy in stage 2
    wmp = np.concatenate([wmp, np.roll(wmp, -64, axis=0)],
                         axis=1).astype(ml_dtypes.bfloat16)

    # stationary slabs [128, 9*DP]: one slab per tap, duplicated into
    # both row halves for the K=64 row-group matmuls
    dt_ = dic.transpose(1, 0, 2, 3)                       # [cin, d, kh, kw]
    wstk = np.zeros((128, 9 * DP), np.float32)
    for k, (kh, kw) in enumerate(TAPS):
        wstk[0:64, k * DP:k * DP + D] = dt_[:, :, kh, kw]
        wstk[64:128, k * DP:k * DP + D] = dt_[:, :, kh, kw]
    wstk = wstk.astype(ml_dtypes.bfloat16)

    xpad = np.zeros((B, CIN, PH, PW), np.float32)
    xpad[:, :, 1:H + 1, 1:W + 1] = x
    xpad = xpad.astype(ml_dtypes.bfloat16)

    in_maps = []
    for c in range(NCORES):
        xf = xpad[c * BPC:(c + 1) * BPC].transpose(1, 0, 2, 3).reshape(CIN, F)
        xxk = np.zeros((128, FX), ml_dtypes.bfloat16)
        xxk[0:64, 0:F] = xf
        xxk[64:128, 0:F - PW - 1] = xf[:, PW + 1:]     # (+1 row, +1 col)
        in_maps.append({
            "xx": np.ascontiguousarray(xxk),
            "wst": wstk, "wm": wmp,
        })
    return in_maps


def _run(in_maps, trace=False, **kw):
    nc = _get_nc()
    return run_bass_kernel_spmd(nc, in_maps, core_ids=list(range(NCORES)),
                                trace=trace, **kw)


def kernel(x, dictionary, lookup_coefficients, lookup_indices):
    in_maps = _prep_inputs(x, dictionary, lookup_coefficients, lookup_indices)
    res = _run(in_maps)
    outs = [np.asarray(res.results[c]["out"]).astype(np.float32)
            for c in range(NCORES)]
    return np.concatenate(outs, axis=0)
